# revision 22
# baseline (speedup 1.0000x reference)
"""Trainium2 Bass kernel for DownNet icosphere GNN (gather-conv + BN + LReLU).

Sharding: data-parallel over batch (B=8 -> 1 batch element per core).
Per conv layer (per core):
  - neighbor-gather feat rows from DRAM h via indirect DMA (gpsimd)
  - PE-transpose gathered [v, kc] tiles into [kc, v] chunks
  - PE matmul with Wmat chunks -> z [v, o] (PSUM accumulate over kc chunks)
  - BN batch stats via ones-matmul column sums (+ sum of squares), tiny
    AllReduce across the 8 cores, then fused affine+LeakyReLU pass
Pool layers: 19-way indirect gather-accumulate (mean folded into next W).
Head: elementwise dot with w_out on-chip, partition-reduce via matmul.
"""

import numpy as np

import concourse.bacc as bacc
import concourse.bass as bass
import concourse.tile as tile
from concourse import mybir
from concourse.bass import IndirectOffsetOnAxis
from concourse.bass_utils import run_bass_kernel_spmd
from concourse.masks import make_identity

F32 = mybir.dt.float32
I32 = mybir.dt.int32
AX = mybir.AxisListType
ALU = mybir.AluOpType

# ---------------------------------------------------------------------------
# configuration


class Cfg:
    def __init__(self, VS, CHS, K, B, n_cores, bn_eps=1e-5, slope=0.2):
        self.VS, self.CHS, self.K, self.B = VS, CHS, K, B
        self.n_cores = n_cores
        self.bn_eps, self.slope = bn_eps, slope
        self.ntiles = [(v + 127) // 128 for v in VS]
        self.vpad = [t * 128 for t in self.ntiles]

    # gather group size (v-tiles per indirect DMA call) for conv at level l
    def conv_G(self, cin, l):
        kc = self.K * cin
        return max(1, min(16, 4864 // kc, self.vpad[l] // self.K))


FULL = Cfg([40962, 10242, 2562, 642], [4, 32, 64, 128, 256], 19, 8, 8)


def _chunks(kc):
    """kc-chunk windows [(start, rows)] covering 0..kc with 128-row windows.
    Last window overlaps backwards; host zeroes the overlapped weight rows."""
    if kc <= 128:
        return [(0, kc)]
    out = [(i * 128, 128) for i in range(kc // 128)]
    if kc % 128:
        out.append((kc - 128, 128))
    return out


# ---------------------------------------------------------------------------
# host-side input preparation


def _prep_conv_idx(neigh, V, G, K):
    """[ngroups, 128, G*K] int32: idx[g, p, t*K+k] = neigh[(g*G+t)*128+p, k]."""
    ntiles = (V + 127) // 128
    ngroups = (ntiles + G - 1) // G
    tpad = ngroups * G
    n = np.zeros((tpad * 128, K), np.int32)
    n[:V] = neigh.reshape(V, K).astype(np.int32)
    return np.ascontiguousarray(
        n.reshape(ngroups, G, 128, K).transpose(0, 2, 1, 3).reshape(ngroups, 128, G * K)
    )


def _prep_pool_idx(neigh_fine, V, K):
    """[128, K*T] int32: idx[p, k*T+t] = neigh_fine[(t*128+p)*K + k]."""
    T = (V + 127) // 128
    n = np.zeros((T * 128, K), np.int32)
    n[:V] = neigh_fine[: V * K].reshape(V, K).astype(np.int32)
    n = n.reshape(T, 128, K).transpose(1, 2, 0)  # [128, K, T]
    return np.ascontiguousarray(n.reshape(128, K * T))


def _prep_w(W, scale=1.0):
    """W [Cout, K, Cin] -> chunked lhsT [nchunk, 128, Cout] f32 (zero-padded
    overlap rows per _chunks)."""
    Cout, K, Cin = W.shape
    kc = K * Cin
    wmat = (np.asarray(W, np.float32) * scale).transpose(1, 2, 0).reshape(kc, Cout)
    ch = _chunks(kc)
    out = np.zeros((len(ch), 128, Cout), np.float32)
    prev_end = 0
    for i, (cs, rows) in enumerate(ch):
        w = wmat[cs : cs + rows].copy()
        if cs < prev_end:  # overlap window: zero rows already accumulated
            w[: prev_end - cs] = 0.0
        out[i, :rows] = w
        prev_end = cs + rows
    return out


def prep_inputs(x, y, params, w_out, b_out, neighs, cfg):
    """Returns (shared_map, per_core_maps)."""
    VS, CHS, K, B = cfg.VS, cfg.CHS, cfg.K, cfg.B
    shared = {}
    # conv1 and conv2 have different Cin -> different G; prepare separately
    for l in range(4):
        cin1 = CHS[l]
        cin2 = CHS[l + 1]
        if l > 0:
            shared[f"idxc1_{l}"] = _prep_conv_idx(neighs[l], VS[l], cfg.conv_G(cin1, l), K)
        shared[f"idxc2_{l}"] = _prep_conv_idx(neighs[l], VS[l], cfg.conv_G(cin2, l), K)
    for l in range(1, 4):
        shared[f"idxp_{l}"] = _prep_pool_idx(neighs[l - 1], VS[l], K)
    # weights + gamma/beta
    for l in range(4):
        for j in range(2):
            W, b, gamma, beta = params[l][j]
            scale = (1.0 / K) if (j == 0 and l > 0) else 1.0  # fold pool mean
            shared[f"w_{l}{j}"] = _prep_w(W, scale)
            shared[f"gb_{l}{j}"] = np.concatenate(
                [np.asarray(gamma, np.float32), np.asarray(beta, np.float32)]
            ).reshape(1, -1)
    # masks [128, 5]: col0 = ones, col 1+l = tail mask of level l
    masks = np.zeros((128, 5), np.float32)
    masks[:, 0] = 1.0
    for l in range(4):
        tail = VS[l] - (cfg.ntiles[l] - 1) * 128
        masks[:tail, 1 + l] = 1.0
    shared["masks"] = masks
    # head
    w_out = np.asarray(w_out, np.float32)
    b_out = np.asarray(b_out, np.float32)
    whid = np.zeros((cfg.vpad[3], CHS[4]), np.float32)
    whid[: VS[3]] = w_out[0, : VS[3] * CHS[4]].reshape(VS[3], CHS[4])
    shared["whid"] = whid
    w_y = float(w_out[0, -1])

    per_core = []
    x = np.asarray(x, np.float32)
    y = np.asarray(y, np.float32)
    G0 = cfg.conv_G(CHS[0], 0)
    ng0 = (cfg.ntiles[0] + G0 - 1) // G0
    n0 = np.zeros((ng0 * G0 * 128,), np.int64)
    n0view = neighs[0].reshape(VS[0], K)
    for c in range(cfg.n_cores):
        m = dict(shared)
        xb = np.zeros((cfg.vpad[0], CHS[0]), np.float32)
        xb[: VS[0]] = x[:, :, c]
        m["x"] = xb
        # feat0 im2col in gather-group layout [ngroups, 128, G*K*Cin]
        f0 = np.zeros((ng0 * G0 * 128, K, CHS[0]), np.float32)
        f0[: VS[0]] = xb[n0view]
        f0 = f0.reshape(ng0, G0, 128, K * CHS[0]).transpose(0, 2, 1, 3)
        m["feat0"] = np.ascontiguousarray(
            f0.reshape(ng0, 128, G0 * K * CHS[0])
        )
        m["headc"] = np.array([[w_y * y[c, 0] + b_out[0]]], np.float32)
        per_core.append(m)
    return per_core


# ---------------------------------------------------------------------------
# bass program builder


def build_nc(cfg):
    VS, CHS, K, B = cfg.VS, cfg.CHS, cfg.K, cfg.B
    nc = bacc.Bacc()
    RG = [list(range(cfg.n_cores))]

    # --- I/O declarations
    x_ext = nc.dram_tensor("x", [cfg.vpad[0], CHS[0]], F32, kind="ExternalInput")
    G0 = cfg.conv_G(CHS[0], 0)
    ng0 = (cfg.ntiles[0] + G0 - 1) // G0
    feat0_ext = nc.dram_tensor(
        "feat0", [ng0, 128, G0 * K * CHS[0]], F32, kind="ExternalInput"
    )
    idxc, idxp, wts, gbs = {}, {}, {}, {}
    for l in range(4):
        for j in range(2):
            cin = CHS[l] if j == 0 else CHS[l + 1]
            G = cfg.conv_G(cin, l)
            ng = (cfg.ntiles[l] + G - 1) // G
            if not (l == 0 and j == 0):
                idxc[(l, j)] = nc.dram_tensor(
                    f"idxc{j + 1}_{l}", [ng, 128, G * K], I32, kind="ExternalInput"
                )
            nchunk = len(_chunks(K * cin))
            wts[(l, j)] = nc.dram_tensor(
                f"w_{l}{j}", [nchunk, 128, CHS[l + 1]], F32, kind="ExternalInput"
            )
            gbs[(l, j)] = nc.dram_tensor(
                f"gb_{l}{j}", [1, 2 * CHS[l + 1]], F32, kind="ExternalInput"
            )
    for l in range(1, 4):
        idxp[l] = nc.dram_tensor(
            f"idxp_{l}", [128, K * cfg.ntiles[l]], I32, kind="ExternalInput"
        )
    masks_ext = nc.dram_tensor("masks", [128, 5], F32, kind="ExternalInput")
    whid_ext = nc.dram_tensor("whid", [cfg.vpad[3], CHS[4]], F32, kind="ExternalInput")
    headc_ext = nc.dram_tensor("headc", [1, 1], F32, kind="ExternalInput")
    out_ext = nc.dram_tensor("out", [1, 1], F32, kind="ExternalOutput")

    with tile.TileContext(nc) as tc:
        with (
            tc.tile_pool(name="singles", bufs=1) as singles,
            tc.tile_pool(name="rows", bufs=2) as rows,
            tc.tile_pool(name="idx", bufs=2) as idxpool,
            tc.tile_pool(name="idxp", bufs=1) as idxppool,
            tc.tile_pool(name="gather", bufs=2) as gpool,
            tc.tile_pool(name="ft", bufs=6) as ftpool,
            tc.tile_pool(name="wsb", bufs=1) as wpool,
            tc.tile_pool(name="zslab", bufs=1) as zpool,
            tc.tile_pool(name="scr", bufs=2) as spool,
            tc.tile_pool(name="pt", bufs=4, space="PSUM") as ptpool,
            tc.tile_pool(name="zp", bufs=2, space="PSUM") as zppool,
            tc.tile_pool(name="stats", bufs=1, space="PSUM") as stpool,
            tc.tile_pool(name="dram", bufs=1, space="DRAM") as dpool,
        ):
            ident = singles.tile([128, 128], F32)
            make_identity(nc, ident[:])
            masks_sb = singles.tile([128, 5], F32)
            nc.sync.dma_start(out=masks_sb[:], in_=masks_ext[:, :])
            eps_sb = singles.tile([128, 1], F32)
            nc.vector.memset(eps_sb[:], cfg.bn_eps)
            headc_sb = singles.tile([1, 1], F32)
            nc.sync.dma_start(out=headc_sb[:], in_=headc_ext[:, :])

            def bcast_dram(dram_ap, parts):
                # DRAM [1, C] row -> [parts, C] via 0-stride partition dim
                return bass.AP(
                    tensor=dram_ap.tensor,
                    offset=dram_ap.offset,
                    ap=[[0, parts]] + [list(d) for d in dram_ap.ap[1:]],
                )

            def gather_src(src_ap, reps):
                # inflate the static source AP with a 0-stride leading dim so
                # the verifier's walk-size bounds check passes; axis=1 keeps
                # the per-index coefficient equal to the row length.
                return bass.AP(
                    tensor=src_ap.tensor,
                    offset=src_ap.offset,
                    ap=[[0, reps]] + [list(d) for d in src_ap.ap],
                )

            def rep_free(ap2d, reps):
                # SBUF [P, C] tile -> [P, reps, C] via 0-stride middle free dim
                return bass.AP(
                    tensor=ap2d.tensor,
                    offset=ap2d.offset,
                    ap=[list(ap2d.ap[0]), [0, reps], list(ap2d.ap[-1])],
                )

            # DRAM intermediates
            hbuf = {}

            def dram_buf(name, V, C):
                t = dpool.tile([V, C], F32, tag=name, name=name)
                hbuf[name] = t
                return t

            def emit_conv(l, j, src, dst, first_layer_src_ext=False):
                cin = CHS[l] if j == 0 else CHS[l + 1]
                cout = CHS[l + 1]
                V = VS[l]
                nt = cfg.ntiles[l]
                G = cfg.conv_G(cin, l)
                ng = (nt + G - 1) // G
                kc = K * cin
                ch = _chunks(kc)
                nchunk = len(ch)

                w_sb = wpool.tile([128, nchunk, cout], F32)
                nc.sync.dma_start(
                    out=w_sb[:], in_=wts[(l, j)][:, :, :].rearrange("n p c -> p n c")
                )
                gb_sb = rows.tile([128, 2 * cout], F32, tag="gb")
                nc.sync.dma_start(out=gb_sb[:], in_=bcast_dram(gbs[(l, j)][:, :], 128))

                z_sb = zpool.tile([128, nt, cout], F32, tag="zslab")
                stats_ps = stpool.tile([1, 2 * cout], F32, tag="stats")

                pregathered = l == 0 and j == 0
                for g in range(ng):
                    g8 = gpool.tile([128, G * K * cin], F32, tag="gather")
                    if pregathered:
                        nc.sync.dma_start(out=g8[:], in_=feat0_ext[g, :, :])
                    else:
                        idx_sb = idxpool.tile([128, G * K], I32, tag="idx")
                        nc.sync.dma_start(out=idx_sb[:], in_=idxc[(l, j)][g, :, :])
                        for tt in range(G):
                            if g * G + tt >= nt:
                                break
                            for k in range(K):
                                c0 = (tt * K + k) * cin
                                nc.gpsimd.indirect_dma_start(
                                    out=g8[:, c0 : c0 + cin],
                                    out_offset=None,
                                    in_=src[:, :],
                                    in_offset=IndirectOffsetOnAxis(
                                        ap=idx_sb[:, tt * K + k : tt * K + k + 1],
                                        axis=0,
                                    ),
                                )
                    for t in range(G):
                        tg = g * G + t
                        if tg >= nt:
                            break
                        base = t * kc
                        fts = []
                        for ci, (cs, rws) in enumerate(ch):
                            pt = ptpool.tile([128, 128], F32, tag="pt")
                            nc.tensor.transpose(
                                out=pt[:rws, :],
                                in_=g8[:, base + cs : base + cs + rws],
                                identity=ident[:],
                            )
                            ft = ftpool.tile([128, 128], F32, tag="ft")
                            if ci % 2 == 0:
                                nc.vector.tensor_copy(out=ft[:rws, :], in_=pt[:rws, :])
                            else:
                                nc.scalar.activation(
                                    out=ft[:rws, :],
                                    in_=pt[:rws, :],
                                    func=mybir.ActivationFunctionType.Copy,
                                )
                            fts.append((ft, rws))
                        zp = zppool.tile([128, cout], F32, tag="zp")
                        for ci, (ft, rws) in enumerate(fts):
                            nc.tensor.matmul(
                                zp[:, :],
                                lhsT=ft[:rws, :],
                                rhs=w_sb[:rws, ci, :],
                                start=(ci == 0),
                                stop=(ci == nchunk - 1),
                            )
                        nc.scalar.activation(
                            out=z_sb[:, tg, :],
                            in_=zp[:, :],
                            func=mybir.ActivationFunctionType.Copy,
                        )
                        zz = ftpool.tile([128, 2 * cout], F32, tag="zz")
                        nc.vector.tensor_copy(out=zz[:, 0:cout], in_=zp[:, :])
                        nc.vector.tensor_tensor(
                            out=zz[:, cout : 2 * cout],
                            in0=zz[:, 0:cout],
                            in1=zz[:, 0:cout],
                            op=ALU.mult,
                        )
                        mask = (
                            masks_sb[:, 0:1] if tg < nt - 1 else masks_sb[:, 1 + l : 2 + l]
                        )
                        nc.tensor.matmul(
                            stats_ps[0:1, 0 : 2 * cout],
                            lhsT=mask,
                            rhs=zz[:, :],
                            start=(tg == 0),
                            stop=(tg == nt - 1),
                            skip_group_check=True,
                        )

                # ---- stats allreduce
                st = rows.tile([1, 2 * cout], F32, tag="st")
                nc.vector.tensor_copy(out=st[:], in_=stats_ps[0:1, :])
                ar_in = dpool.tile([1, 2 * cout], F32, tag=f"arin_{l}{j}")
                ar_out = dpool.tile([1, 2 * cout], F32, tag=f"arout_{l}{j}")
                nc.sync.dma_start(out=ar_in[:], in_=st[:])
                nc.gpsimd.collective_compute(
                    "AllReduce",
                    ALU.add,
                    replica_groups=RG,
                    ins=[ar_in[:]],
                    outs=[ar_out[:]],
                )
                st2 = rows.tile([128, 2 * cout], F32, tag="st2")
                nc.sync.dma_start(out=st2[:], in_=bcast_dram(ar_out[:], 128))

                # ---- bn coefficients a, b (replicated across 128 partitions)
                invN = 1.0 / (V * B)
                mean = rows.tile([128, cout], F32, tag="mean")
                nc.vector.tensor_scalar_mul(mean[:], st2[:, 0:cout], invN)
                ex2 = rows.tile([128, cout], F32, tag="ex2")
                nc.vector.tensor_scalar_mul(ex2[:], st2[:, cout : 2 * cout], invN)
                var = rows.tile([128, cout], F32, tag="var")
                nc.vector.tensor_tensor(out=var[:], in0=mean[:], in1=mean[:], op=ALU.mult)
                nc.vector.tensor_tensor(out=var[:], in0=ex2[:], in1=var[:], op=ALU.subtract)
                sd = rows.tile([128, cout], F32, tag="sd")
                nc.scalar.activation(
                    out=sd[:],
                    in_=var[:],
                    func=mybir.ActivationFunctionType.Sqrt,
                    bias=eps_sb[:, 0:1],
                )
                rr = rows.tile([128, cout], F32, tag="rr")
                nc.vector.reciprocal(out=rr[:], in_=sd[:])
                a_row = rows.tile([128, cout], F32, tag="a_row")
                nc.vector.tensor_tensor(
                    out=a_row[:], in0=gb_sb[:, 0:cout], in1=rr[:], op=ALU.mult
                )
                b_row = rows.tile([128, cout], F32, tag="b_row")
                nc.vector.tensor_tensor(
                    out=b_row[:], in0=mean[:], in1=a_row[:], op=ALU.mult
                )
                nc.vector.tensor_tensor(
                    out=b_row[:],
                    in0=gb_sb[:, cout : 2 * cout],
                    in1=b_row[:],
                    op=ALU.subtract,
                )

                # ---- bn apply + leaky relu (chunked scratch to save SBUF)
                a_b = rep_free(a_row[:], nt)
                b_b = rep_free(b_row[:], nt)
                nc.vector.tensor_tensor(out=z_sb[:], in0=z_sb[:], in1=a_b, op=ALU.mult)
                nc.vector.tensor_tensor(out=z_sb[:], in0=z_sb[:], in1=b_b, op=ALU.add)
                dst_v = dst[:, :].rearrange("(t p) c -> p t c", p=128)
                CH = max(1, 2048 // cout)
                for s in range(0, nt, CH):
                    e = min(s + CH, nt)
                    scr = spool.tile([128, CH, cout], F32, tag="scr")
                    nc.vector.tensor_scalar_mul(
                        scr[:, : e - s, :], z_sb[:, s:e, :], cfg.slope
                    )
                    nc.vector.tensor_tensor(
                        out=scr[:, : e - s, :],
                        in0=z_sb[:, s:e, :],
                        in1=scr[:, : e - s, :],
                        op=ALU.max,
                    )
                    nc.sync.dma_start(out=dst_v[:, s:e, :], in_=scr[:, : e - s, :])

            def emit_pool(l, src, dst):
                C = CHS[l]  # channels entering level l (= CHS[l].. conv1 input)
                T = cfg.ntiles[l]
                idx_sb = idxppool.tile([128, K * T], I32, tag="idxp")
                nc.sync.dma_start(out=idx_sb[:], in_=idxp[l][:, :])
                acc = zpool.tile([128, T, C], F32, tag="zslab")
                for k in range(K):
                    for tt in range(T):
                        nc.gpsimd.indirect_dma_start(
                            out=acc[:, tt, :],
                            out_offset=None,
                            in_=src[:, :],
                            in_offset=IndirectOffsetOnAxis(
                                ap=idx_sb[:, k * T + tt : k * T + tt + 1], axis=0
                            ),
                            compute_op=ALU.bypass if k == 0 else ALU.add,
                        )
                nc.sync.dma_start(
                    out=dst[:, :].rearrange("(t p) c -> p t c", p=128), in_=acc[:]
                )

            # ------------- network
            h = {}
            h["c1_0"] = dram_buf("h_c1_0", cfg.vpad[0], CHS[1])
            h["c2_0"] = dram_buf("h_c2_0", cfg.vpad[0], CHS[1])
            emit_conv(0, 0, x_ext, h["c1_0"])
            emit_conv(0, 1, h["c1_0"], h["c2_0"])
            prev = h["c2_0"]
            for l in range(1, 4):
                hp = dram_buf(f"h_p{l}", cfg.vpad[l], CHS[l])
                emit_pool(l, prev, hp)
                hc1 = dram_buf(f"h_c1_{l}", cfg.vpad[l], CHS[l + 1])
                emit_conv(l, 0, hp, hc1)
                hc2 = dram_buf(f"h_c2_{l}", cfg.vpad[l], CHS[l + 1])
                emit_conv(l, 1, hc1, hc2)
                prev = hc2

            # ------------- head
            T3 = cfg.ntiles[3]
            C4 = CHS[4]
            hw = gpool.tile([128, T3, C4], F32, tag="gather")
            nc.sync.dma_start(
                out=hw[:], in_=whid_ext[:, :].rearrange("(t p) c -> p t c", p=128)
            )
            h3 = gpool.tile([128, T3, C4], F32, tag="gather")
            nc.sync.dma_start(
                out=h3[:], in_=prev[:, :].rearrange("(t p) c -> p t c", p=128)
            )
            nc.vector.tensor_tensor(out=h3[:], in0=h3[:], in1=hw[:], op=ALU.mult)
            acc1 = rows.tile([128, 1], F32, tag="acc1")
            nc.vector.tensor_reduce(
                out=acc1[:], in_=h3[:], axis=AX.XY, op=ALU.add
            )
            ps1 = stpool.tile([1, 1], F32, tag="ps1")
            nc.tensor.matmul(
                ps1[0:1, 0:1],
                lhsT=acc1[:],
                rhs=masks_sb[:, 0:1],
                start=True,
                stop=True,
                skip_group_check=True,
            )
            fin = rows.tile([1, 1], F32, tag="fin")
            nc.vector.tensor_copy(out=fin[:], in_=ps1[0:1, :])
            nc.vector.tensor_tensor(
                out=fin[:], in0=fin[:], in1=headc_sb[:], op=ALU.add
            )
            nc.sync.dma_start(out=out_ext[:, :], in_=fin[:])

    nc.compile()
    return nc


# ---------------------------------------------------------------------------
# public entry point

_CACHE = {}


def _get_nc(cfg):
    key = (tuple(cfg.VS), tuple(cfg.CHS), cfg.K, cfg.B, cfg.n_cores)
    if key not in _CACHE:
        _CACHE[key] = build_nc(cfg)
    return _CACHE[key]


def run(x, y, params, w_out, b_out, neighs, cfg, trace=False):
    nc = _get_nc(cfg)
    in_maps = prep_inputs(x, y, params, w_out, b_out, neighs, cfg)
    res = run_bass_kernel_spmd(
        nc, in_maps, core_ids=list(range(cfg.n_cores)), trace=trace
    )
    out = np.array(
        [res.results[c]["out"][0, 0] for c in range(cfg.n_cores)], np.float32
    ).reshape(cfg.n_cores, 1)
    return out, res


def kernel(x, y, params, w_out, b_out, neigh0, neigh1, neigh2, neigh3):
    out, _ = run(x, y, params, w_out, b_out, [neigh0, neigh1, neigh2, neigh3], FULL)
    return out
